# revision 6
# baseline (speedup 1.0000x reference)
"""AdjMultiHeadAttention Trainium2 kernel, v2.

Sharding: pure data-parallel over batch. B=16 over 8 cores -> 2 batches/core.

Per-core design (driven by the TimelineSim cost model):
  - fp8e4 DoubleRow matmuls (0.5 cyc/row, 2 k-tiles/instr) for the scores,
    v-projection and out-projection; bf16 for qk-projection (accuracy) and
    the ctx matmul (mega must stay bf16).
  - scores are computed transposed (scoresT[sk,sq]) with q,k in fp8 produced
    for free by the projection PSUM drains. The fp8 DR scores matmul uses a
    zero-padded second k-tile in the stationary operand (contraction is only
    d=64) -- cost is halved vs bf16 regardless.
  - The elementwise wall (mask-multiply + exp on 16.8M f32 PSUM elements per
    core) is split three ways:
      * DV of 8 sk-tiles per (u,hh): DVE drains with the mask fused
        (scalar_tensor_tensor, x184.66496 Schraudolph prescale), then one
        cheap 4x-mode tensor_scalar (+16248.577 -> int16) computes exp via
        the Schraudolph bf16-bit trick.
      * the rest: ACT drains with func=Exp fused (exact exp of the raw
        scores), then gpsimd computes E^mask with tensor_tensor(op=pow)
        (exp(s*m) == exp(s)^m), one big op per (u,hh).
  - softmax denominators ride along as 1-wide ones-column matmuls into a tiny
    PSUM tile (PE cost ~0 in the model); normalization is fused into the
    PSUM->SBUF ctx drain via a per-partition-broadcast reciprocal.
  - ctx is transposed 128x64-block-wise on the PE (bf16), drained to fp8 for
    the DR out-projection.
Emission is software-pipelined over units u=(batch, head-pair) with a step
backlog popped between score tiles, as in v1.
"""

import os
import sys

sys.path.insert(0, "/opt/trn_rl_repo")

from contextlib import ExitStack

import ml_dtypes
import numpy as np

import concourse.bass as bass
import concourse.tile as tile
from concourse import bacc, mybir
from concourse.bass_utils import run_bass_kernel_spmd
from concourse.masks import make_identity

B, S, E, H, D = 16, 1024, 512, 8, 64
NCORES = 8
BPC = B // NCORES
SCALE = D**-0.5
BF16 = mybir.dt.bfloat16
F32 = mybir.dt.float32
F8 = mybir.dt.float8e4
I16 = mybir.dt.int16
NPBF16 = ml_dtypes.bfloat16
NPF8 = ml_dtypes.float8_e4m3

# Schraudolph constants for bf16 exp-by-bits: int16bits(bf16(e^x)) ~=
# round(x*184.66496 + 16248.577)
SCH_A = 184.66496
SCH_B = 16248.577

_cache = {}

NU = BPC * 4


def _build(bo_nonzero: bool, bv_nonzero: bool = False, knobs=None):
    knobs = knobs or {}
    WARM = int(knobs.get("warm", os.environ.get("K_WARM", 8)))
    DV = int(knobs.get("dv", os.environ.get("K_DV", 4)))       # DVE-drained sk per (u,hh)
    QKDVE = int(knobs.get("qkdve", os.environ.get("K_QKDVE", 4)))  # of 32 qk drains on DVE
    OUTDVE = int(knobs.get("outdve", os.environ.get("K_OUTDVE", 0)))  # of 16 out drains on DVE
    assert 0 <= DV <= 8

    nc = bacc.Bacc("TRN2", target_bir_lowering=False, debug=False, num_devices=NCORES)

    xT_d = nc.dram_tensor("xT", [BPC, E, S], BF16, kind="ExternalInput").ap()
    maskT_d = nc.dram_tensor("maskT", [BPC, S, S], BF16, kind="ExternalInput").ap()
    wq_d = nc.dram_tensor("wqT", [E, E], BF16, kind="ExternalInput").ap()
    wk_d = nc.dram_tensor("wkT", [E, E], BF16, kind="ExternalInput").ap()
    wv_d = nc.dram_tensor("wvT", [E, E], BF16, kind="ExternalInput").ap()
    wo_d = nc.dram_tensor("woT", [E, E], BF16, kind="ExternalInput").ap()
    bqk_d = nc.dram_tensor("bqk", [128, 8], F32, kind="ExternalInput").ap()
    bv_d = nc.dram_tensor("bv", [E], F32, kind="ExternalInput").ap()
    bo_d = nc.dram_tensor("bo", [E], F32, kind="ExternalInput").ap()
    out_d = nc.dram_tensor("out", [BPC, S, E], F32, kind="ExternalOutput").ap()

    mult = mybir.AluOpType.mult
    add = mybir.AluOpType.add
    powop = mybir.AluOpType.pow
    EXP = mybir.ActivationFunctionType.Exp
    IDENT = mybir.ActivationFunctionType.Identity
    COPY = mybir.ActivationFunctionType.Copy
    DRMODE = mybir.MatmulPerfMode.DoubleRow

    KPAD = S  # zero-pad col offset in k tiles

    with tile.TileContext(nc) as tc, ExitStack() as ctx:
        singles = ctx.enter_context(tc.tile_pool(name="singles", bufs=1))
        xtp = ctx.enter_context(tc.tile_pool(name="xt", bufs=BPC))
        maskp = ctx.enter_context(tc.tile_pool(name="mask", bufs=BPC))
        qp = ctx.enter_context(tc.tile_pool(name="qt", bufs=3))
        kp = ctx.enter_context(tc.tile_pool(name="kt", bufs=3))
        vp = ctx.enter_context(tc.tile_pool(name="v", bufs=8 * BPC))
        megap = ctx.enter_context(tc.tile_pool(name="mega", bufs=3))
        ctxp = ctx.enter_context(tc.tile_pool(name="ctx", bufs=BPC))
        ctxTp = ctx.enter_context(tc.tile_pool(name="ctxT", bufs=BPC))
        outp = ctx.enter_context(tc.tile_pool(name="outs", bufs=3))
        rcp = ctx.enter_context(tc.tile_pool(name="rc", bufs=4))
        scp = ctx.enter_context(tc.tile_pool(name="sc", bufs=2, space="PSUM"))
        pcp = ctx.enter_context(tc.tile_pool(name="pc", bufs=2, space="PSUM"))
        mmp = ctx.enter_context(tc.tile_pool(name="mm", bufs=2, space="PSUM"))

        # ---- constants ----
        w_sb = {}

        def load_w(name, d, dt):
            t = singles.tile([128, 4 * E], dt, tag=f"w{name}", name=f"w{name}")
            ov = t[:].rearrange("p (c f) -> p c f", c=4)
            iv = d.rearrange("(c p) f -> p c f", p=128)
            nc.sync.dma_start(out=ov, in_=iv)
            w_sb[name] = t

        bqk_sb = singles.tile([128, 8], F32, tag="bqk")
        nc.sync.dma_start(out=bqk_sb[:], in_=bqk_d[:])
        ident = singles.tile([128, 128], BF16, tag="ident")
        make_identity(nc, ident[:])
        bv_sb = None
        if bv_nonzero:
            bv_sb = singles.tile([128, E], F32, tag="bv")
            nc.sync.dma_start(
                out=bv_sb[:],
                in_=bass.AP(tensor=bv_d.tensor, offset=bv_d.offset,
                            ap=[[0, 128]] + bv_d.ap),
            )
        bo_sb = None
        if bo_nonzero:
            bo_sb = singles.tile([128, E], F32, tag="bo")
            nc.sync.dma_start(
                out=bo_sb[:],
                in_=bass.AP(tensor=bo_d.tensor, offset=bo_d.offset,
                            ap=[[0, 128]] + bo_d.ap),
            )
        warm_in = singles.tile([128, 512], BF16, tag="warm")
        nc.gpsimd.memset(warm_in[:], 0.0)
        warm_ps = mmp.tile([128, 512], F32, tag="mm", name="warmps")
        for _ in range(WARM):
            nc.tensor.matmul(warm_ps[:], lhsT=ident[:], rhs=warm_in[:],
                             start=True, stop=True)

        # ---- pipeline state ----
        xt = {}
        masks = {}
        qk = {}       # (b, 'q'|'k', j) -> fp8 tile
        v_sb = {}     # b -> [8 tiles]
        mega = {}     # (u, hh) -> bf16 tile [128, 8S]
        ctx_sb = {}   # b -> [128, 4096] bf16
        ctxT = {}     # b -> [128, 4096] fp8
        rc_t = {}     # u -> [128, 16] f32
        pd_t = {}     # u -> psum [128, 16]
        drain_ct = {"qk": 0, "out": 0}

        def dma_in_x(b, half=None):
            if b in xt:
                t = xt[b]
            else:
                t = xtp.tile([128, 4 * S], BF16, tag="xt", name=f"xt{b}")
                xt[b] = t
            ov = t[:].rearrange("p (e s) -> p e s", e=4)
            iv = xT_d[b].rearrange("(e p) s -> p e s", p=128)
            if half in (None, 0):
                nc.sync.dma_start(out=ov[:, 0:2], in_=iv[:, 0:2])
            if half in (None, 1):
                nc.sync.dma_start(out=ov[:, 2:4], in_=iv[:, 2:4])

        def dma_in_mask(b, pieces=((0, 4), (4, 8))):
            if b in masks:
                t = masks[b]
            else:
                t = maskp.tile([128, 8 * S], BF16, tag="mask", name=f"mask{b}")
                masks[b] = t
            ov = t[:].rearrange("p (sk sq) -> p sk sq", sk=8)
            iv = maskT_d[b].rearrange("(sk p) sq -> p sk sq", p=128)
            for lo, hi in pieces:
                nc.sync.dma_start(out=ov[:, lo:hi], in_=iv[:, lo:hi])

        def dma_in(b):
            dma_in_x(b)
            dma_in_mask(b)

        def dr_ap(t, prow, nrow, col, stride2, ncol):
            """[nrow part @ prow, 2 @ stride2, ncol] view of tile t."""
            base = t[prow:prow + nrow, col:col + 1]
            return bass.AP(tensor=base.tensor, offset=base.offset,
                           ap=[base.ap[0], [stride2, 2], [1, ncol]])

        def qk_drain(ps, dst, col, sh):
            """PSUM [128,512] -> fp8 q/k slice with bias."""
            i = drain_ct["qk"]
            drain_ct["qk"] += 1
            osl = dst[:, sh * 512:(sh + 1) * 512]
            if i % 8 < (QKDVE + 3) // 4:
                nc.vector.tensor_scalar(osl, ps[:], bqk_sb[:, col:col + 1], None, add)
            else:
                nc.scalar.activation(osl, ps[:], IDENT,
                                     bias=bqk_sb[:, col:col + 1], scale=1.0)

        def proj_qk_steps(b, j):
            steps = []
            qt = qp.tile([128, S], BF16, tag="qt", name=f"q{b}_{j}")
            kt = kp.tile([128, S], BF16, tag="kt", name=f"k{b}_{j}")
            qk[(b, "q", j)] = qt
            qk[(b, "k", j)] = kt
            for ti, (tname, dst) in enumerate((("q", qt), ("k", kt))):
                col = ti * 4 + j
                wname = tname
                for sh in range(2):
                    def mk(tname=wname, dst=dst, col=col, sh=sh):
                        def step():
                            ps = mmp.tile([128, 512], F32, tag="mm",
                                          name=f"pqk{b}{j}{tname}{sh}")
                            for e in range(4):
                                nc.tensor.matmul(
                                    ps[:],
                                    lhsT=w_sb[tname][:, e * E + j * 128: e * E + (j + 1) * 128],
                                    rhs=xt[b][:, e * S + sh * 512: e * S + (sh + 1) * 512],
                                    start=(e == 0), stop=(e == 3),
                                )
                            qk_drain(ps, dst, col, sh)
                        return step
                    steps.append(mk())
            return steps

        def proj_v_steps(b):
            v_sb[b] = [None] * 8
            steps = []

            def mk(s):
                def step():
                    ps = mmp.tile([128, 512], F32, tag="mm", name=f"pv{b}_{s}")
                    for e in range(4):
                        nc.tensor.matmul(
                            ps[:],
                            lhsT=xt[b][:, e * S + s * 128: e * S + (s + 1) * 128],
                            rhs=w_sb["v"][:, e * E:(e + 1) * E],
                            start=(e == 0), stop=(e == 3),
                        )
                    vt = vp.tile([128, 8 * 65], BF16, tag="v", name=f"v{b}_{s}")
                    vv = vt[:].rearrange("p (h c) -> p h c", h=8)
                    pv = ps[:].rearrange("p (h c) -> p h c", h=8)
                    if bv_nonzero:
                        nc.vector.scalar_tensor_tensor(
                            out=vv[:, :, 0:64], in0=pv[:, :, :], scalar=1.0,
                            in1=bv_sb[:].rearrange("p (h c) -> p h c", h=8),
                            op0=mult, op1=add)
                    else:
                        nc.scalar.activation(vv[:, :, 0:64], pv[:, :, :], COPY)
                    nc.gpsimd.memset(vv[:, :, 64:65], 1.0)
                    v_sb[b][s] = vt
                return step

            for s in range(8):
                steps.append(mk(s))
            return steps

        def attn_a(u, bsteps):
            b, j = divmod(u, 4)
            kt = qk[(b, "k", j)]
            qt = qk[(b, "q", j)]
            slot = 0
            nslots = 16
            for hh in range(2):
                mg = megap.tile([128, 8 * S], BF16, tag="mega", name=f"mega{u}_{hh}")
                mega[(u, hh)] = mg
                for sk in range(8):
                    ps = scp.tile([128, S], F32, tag="sc", name=f"sc{u}{hh}{sk}")
                    for sh in range(2):
                        nc.tensor.matmul(
                            ps[:, sh * 512:(sh + 1) * 512],
                            lhsT=kt[hh * 64: hh * 64 + 64, sk * 128:(sk + 1) * 128],
                            rhs=qt[hh * 64: hh * 64 + 64, sh * 512:(sh + 1) * 512],
                            start=True, stop=True,
                        )
                    if sk < DV:
                        nc.vector.scalar_tensor_tensor(
                            out=mg[:, sk * S:(sk + 1) * S],
                            in0=ps[:], scalar=SCH_A,
                            in1=masks[b][:, sk * S:(sk + 1) * S],
                            op0=mult, op1=mult,
                        )
                    else:
                        nc.scalar.activation(mg[:, sk * S:(sk + 1) * S], ps[:], EXP)
                    slots_left = nslots - slot
                    n = (len(bsteps) + slots_left - 1) // slots_left if bsteps else 0
                    for _ in range(n):
                        if bsteps:
                            bsteps.pop(0)()
                    slot += 1
                if DV > 0:
                    mgi = mg[:].bitcast(I16)
                    nc.vector.tensor_scalar(
                        mgi[:, 0:DV * S], mg[:, 0:DV * S], 1.0, SCH_B, mult, add)
                if DV < 8:
                    nc.gpsimd.tensor_tensor(
                        out=mg[:, DV * S:8 * S],
                        in0=mg[:, DV * S:8 * S],
                        in1=masks[b][:, DV * S:8 * S],
                        op=powop,
                    )
            for st in bsteps:
                st()

        def ctx_steps(u):
            b, j = divmod(u, 4)
            steps = []
            if b not in ctx_sb:
                ctx_sb[b] = ctxp.tile([128, 4096], BF16, tag="ctx", name=f"ctx{b}")

            def mk_quarter(q4):
                def step():
                    pc = pcp.tile([128, 260], F32, tag="pc", name=f"pc{u}_{q4}")
                    for i in range(2):
                        sq = q4 * 2 + i
                        for hh in range(2):
                            h = 2 * j + hh
                            mg = mega[(u, hh)]
                            for sk in range(8):
                                nc.tensor.matmul(
                                    pc[:, (i * 2 + hh) * 64:(i * 2 + hh) * 64 + 64],
                                    lhsT=mg[:, sk * S + sq * 128: sk * S + sq * 128 + 128],
                                    rhs=v_sb[b][sk][:, h * 65: h * 65 + 64],
                                    start=(sk == 0), stop=(sk == 7),
                                )
                            for sk in range(8):
                                nc.tensor.matmul(
                                    pc[:, 256 + i * 2 + hh: 257 + i * 2 + hh],
                                    lhsT=mg[:, sk * S + sq * 128: sk * S + sq * 128 + 128],
                                    rhs=v_sb[b][sk][:, h * 65 + 64: h * 65 + 65],
                                    start=(sk == 0), stop=(sk == 7),
                                )
                    rcq = rcp.tile([128, 4], F32, tag="rc", name=f"rc{u}_{q4}")
                    nc.vector.reciprocal(rcq[:], pc[:, 256:260])
                    in0 = pc[:, 0:256].rearrange("p (i hh d) -> p i hh d", i=2, hh=2)
                    base = rcq[0:128, 0:1]
                    in1 = bass.AP(tensor=base.tensor, offset=base.offset,
                                  ap=[base.ap[0], [2, 2], [1, 2], [0, 64]])
                    ov = ctx_sb[b][:].rearrange(
                        "p (sq h d) -> p sq h d", sq=8, h=8
                    )[:, q4 * 2:(q4 + 1) * 2, 2 * j:2 * j + 2, :]
                    nc.vector.scalar_tensor_tensor(
                        out=ov, in0=in0, scalar=1.0, in1=in1, op0=mult, op1=mult)
                return step

            for q4 in range(4):
                steps.append(mk_quarter(q4))
            return steps

        def transpose_steps(u):
            b, j = divmod(u, 4)
            steps = []
            if b not in ctxT:
                ctxT[b] = ctxTp.tile([128, 4096], BF16, tag="ctxT", name=f"ctxT{b}")

            def mk_tr(sq4):
                def step():
                    pt = mmp.tile([128, 512], BF16, tag="mm", name=f"pt{u}_{sq4}")
                    for hh in range(2):
                        h = 2 * j + hh
                        for sqi in range(4):
                            sq = sq4 * 4 + sqi
                            nc.tensor.transpose(
                                out=pt[hh * 64: hh * 64 + 64, sqi * 128:(sqi + 1) * 128],
                                in_=ctx_sb[b][:, sq * 512 + h * 64: sq * 512 + h * 64 + 64],
                                identity=ident[:],
                            )
                    nc.scalar.activation(
                        ctxT[b][:, j * S + sq4 * 512: j * S + (sq4 + 1) * 512],
                        pt[:], COPY)
                return step

            for sq4 in range(2):
                steps.append(mk_tr(sq4))
            return steps

        def outproj_steps(b, half):
            steps = []

            def mk(si):
                def step():
                    s = half * 4 + si
                    po = mmp.tile([128, 512], F32, tag="mm", name=f"po{b}_{s}")
                    for j4 in range(4):
                        nc.tensor.matmul(
                            po[:],
                            lhsT=ctxT[b][:, j4 * S + s * 128: j4 * S + (s + 1) * 128],
                            rhs=w_sb["o"][:, j4 * E:(j4 + 1) * E],
                            start=(j4 == 0), stop=(j4 == 3),
                        )
                    ou = outp.tile([128, 512], F32, tag="outs", name=f"ou{b}_{s}")
                    i = drain_ct["out"]
                    drain_ct["out"] += 1
                    if bo_nonzero:
                        nc.vector.scalar_tensor_tensor(
                            out=ou[:], in0=po[:], scalar=1.0, in1=bo_sb[:],
                            op0=mult, op1=add)
                    elif i % 16 < OUTDVE:
                        nc.vector.tensor_scalar(ou[:], po[:], 1.0, None, mult)
                    else:
                        nc.scalar.activation(ou[:], po[:], COPY)
                    nc.sync.dma_start(
                        out=out_d[b, s * 128:(s + 1) * 128, :], in_=ou[:])
                return step

            for si in range(4):
                steps.append(mk(si))
            return steps

        # ---- emission ----
        load_w("q", wq_d, BF16)
        dma_in_x(0)
        load_w("k", wk_d, BF16)
        dma_in_mask(0, pieces=((0, 1), (1, 2), (2, 4), (4, 8)))
        load_w("v", wv_d, BF16)
        load_w("o", wo_d, BF16)

        for st in proj_qk_steps(0, 0):
            st()
        for st in proj_v_steps(0):
            st()

        for u in range(NU):
            b, j = divmod(u, 4)
            if u == 1 and BPC > 1:
                dma_in(1)
            bsteps = []
            if u + 1 < NU:
                nb, nj = divmod(u + 1, 4)
                bsteps += proj_qk_steps(nb, nj)
            if u == 3 and BPC > 1:
                bsteps += proj_v_steps(1)
            if u >= 1:
                bsteps += ctx_steps(u - 1)
            if u >= 2:
                bsteps += transpose_steps(u - 2)
            if u == 5:
                bsteps += outproj_steps(0, 0)
            if u == 6:
                bsteps += outproj_steps(0, 1)
            attn_a(u, bsteps)

        # tail
        tail = []
        tail += ctx_steps(NU - 1)
        tail += transpose_steps(NU - 2)
        for st in tail:
            st()
        for st in transpose_steps(NU - 1):
            st()
        for st in outproj_steps(BPC - 1, 0) + outproj_steps(BPC - 1, 1):
            st()

    nc.compile()
    return nc


def _prep(x, adj_matrix, bond_matrix, Wq, bq, Wk, bk, Wv, bv, Wo, bo):
    x = np.asarray(x, np.float32)
    mask = np.asarray(adj_matrix, np.float32) + np.asarray(bond_matrix, np.float32)
    xT = np.ascontiguousarray(x.transpose(0, 2, 1))
    xTb = xT.astype(NPBF16)
    maskT = np.ascontiguousarray(mask.transpose(0, 2, 1)).astype(NPBF16)
    wqT = np.ascontiguousarray(np.asarray(Wq, np.float32).T * SCALE).astype(NPBF16)
    wkT = np.ascontiguousarray(np.asarray(Wk, np.float32).T).astype(NPBF16)
    wvT = np.ascontiguousarray(np.asarray(Wv, np.float32).T).astype(NPBF16)
    woT = np.ascontiguousarray(np.asarray(Wo, np.float32).T).astype(NPBF16)
    bqs = np.asarray(bq, np.float32) * SCALE
    bkf = np.asarray(bk, np.float32)
    bqk = np.concatenate(
        [bqs.reshape(4, 128).T, bkf.reshape(4, 128).T], axis=1).astype(np.float32)
    bqk = np.ascontiguousarray(bqk)
    bvf = np.ascontiguousarray(np.asarray(bv, np.float32))
    bof = np.ascontiguousarray(np.asarray(bo, np.float32))

    in_maps = []
    for c in range(NCORES):
        sl = slice(c * BPC, (c + 1) * BPC)
        in_maps.append({
            "xT": np.ascontiguousarray(xTb[sl]),
            "maskT": np.ascontiguousarray(maskT[sl]),
            "wqT": wqT, "wkT": wkT, "wvT": wvT, "woT": woT,
            "bqk": bqk, "bv": bvf, "bo": bof,
        })
    return in_maps, bool(np.any(bof)), bool(np.any(bvf))


def kernel(x, adj_matrix, bond_matrix, Wq, bq, Wk, bk, Wv, bv, Wo, bo,
           seq_len, _trace=False, _knobs=None):
    in_maps, bo_nonzero, bv_nonzero = _prep(
        x, adj_matrix, bond_matrix, Wq, bq, Wk, bk, Wv, bv, Wo, bo)
    key = ("k", bo_nonzero, bv_nonzero, str(_knobs))
    if key not in _cache:
        _cache[key] = _build(bo_nonzero, bv_nonzero, _knobs)
    nc = _cache[key]
    res = run_bass_kernel_spmd(
        nc, in_maps, core_ids=list(range(NCORES)), trace=_trace)
    out = np.concatenate([r["out"] for r in res.results], axis=0).astype(np.float32)
    if _trace:
        kernel._last_exec_time_ns = res.exec_time_ns
        kernel._last_results = res
    return out


# revision 7
# speedup vs baseline: 1.0019x; 1.0019x over previous
"""AdjMultiHeadAttention Trainium2 kernel, v2.

Sharding: pure data-parallel over batch. B=16 over 8 cores -> 2 batches/core.

Per-core design (driven by the TimelineSim cost model):
  - fp8e4 DoubleRow matmuls (0.5 cyc/row, 2 k-tiles/instr) for the scores,
    v-projection and out-projection; bf16 for qk-projection (accuracy) and
    the ctx matmul (mega must stay bf16).
  - scores are computed transposed (scoresT[sk,sq]) with q,k in fp8 produced
    for free by the projection PSUM drains. The fp8 DR scores matmul uses a
    zero-padded second k-tile in the stationary operand (contraction is only
    d=64) -- cost is halved vs bf16 regardless.
  - The elementwise wall (mask-multiply + exp on 16.8M f32 PSUM elements per
    core) is split three ways:
      * DV of 8 sk-tiles per (u,hh): DVE drains with the mask fused
        (scalar_tensor_tensor, x184.66496 Schraudolph prescale), then one
        cheap 4x-mode tensor_scalar (+16248.577 -> int16) computes exp via
        the Schraudolph bf16-bit trick.
      * the rest: ACT drains with func=Exp fused (exact exp of the raw
        scores), then gpsimd computes E^mask with tensor_tensor(op=pow)
        (exp(s*m) == exp(s)^m), one big op per (u,hh).
  - softmax denominators ride along as 1-wide ones-column matmuls into a tiny
    PSUM tile (PE cost ~0 in the model); normalization is fused into the
    PSUM->SBUF ctx drain via a per-partition-broadcast reciprocal.
  - ctx is transposed 128x64-block-wise on the PE (bf16), drained to fp8 for
    the DR out-projection.
Emission is software-pipelined over units u=(batch, head-pair) with a step
backlog popped between score tiles, as in v1.
"""

import os
import sys

sys.path.insert(0, "/opt/trn_rl_repo")

from contextlib import ExitStack

import ml_dtypes
import numpy as np

import concourse.bass as bass
import concourse.tile as tile
from concourse import bacc, mybir
from concourse.bass_utils import run_bass_kernel_spmd
from concourse.masks import make_identity

B, S, E, H, D = 16, 1024, 512, 8, 64
NCORES = 8
BPC = B // NCORES
SCALE = D**-0.5
BF16 = mybir.dt.bfloat16
F32 = mybir.dt.float32
F8 = mybir.dt.float8e4
I16 = mybir.dt.int16
NPBF16 = ml_dtypes.bfloat16
NPF8 = ml_dtypes.float8_e4m3

# Schraudolph constants for bf16 exp-by-bits: int16bits(bf16(e^x)) ~=
# round(x*184.66496 + 16248.577)
SCH_A = 184.66496
SCH_B = 16248.577

_cache = {}

NU = BPC * 4


def _build(bo_nonzero: bool, bv_nonzero: bool = False, knobs=None):
    knobs = knobs or {}
    WARM = int(knobs.get("warm", os.environ.get("K_WARM", 8)))
    DV = int(knobs.get("dv", os.environ.get("K_DV", 4)))       # DVE-drained sk per (u,hh)
    QKDVE = int(knobs.get("qkdve", os.environ.get("K_QKDVE", 4)))  # of 32 qk drains on DVE
    OUTDVE = int(knobs.get("outdve", os.environ.get("K_OUTDVE", 0)))  # of 16 out drains on DVE
    assert 0 <= DV <= 8

    nc = bacc.Bacc("TRN2", target_bir_lowering=False, debug=False, num_devices=NCORES)

    xT_d = nc.dram_tensor("xT", [BPC, E, S], BF16, kind="ExternalInput").ap()
    maskT_d = nc.dram_tensor("maskT", [BPC, S, S], BF16, kind="ExternalInput").ap()
    wq_d = nc.dram_tensor("wqT", [E, E], BF16, kind="ExternalInput").ap()
    wk_d = nc.dram_tensor("wkT", [E, E], BF16, kind="ExternalInput").ap()
    wv_d = nc.dram_tensor("wvT", [E, E], BF16, kind="ExternalInput").ap()
    wo_d = nc.dram_tensor("woT", [E, E], BF16, kind="ExternalInput").ap()
    bqk_d = nc.dram_tensor("bqk", [128, 8], F32, kind="ExternalInput").ap()
    bv_d = nc.dram_tensor("bv", [E], F32, kind="ExternalInput").ap()
    bo_d = nc.dram_tensor("bo", [E], F32, kind="ExternalInput").ap()
    out_d = nc.dram_tensor("out", [BPC, S, E], F32, kind="ExternalOutput").ap()

    mult = mybir.AluOpType.mult
    add = mybir.AluOpType.add
    powop = mybir.AluOpType.pow
    EXP = mybir.ActivationFunctionType.Exp
    IDENT = mybir.ActivationFunctionType.Identity
    COPY = mybir.ActivationFunctionType.Copy
    DRMODE = mybir.MatmulPerfMode.DoubleRow

    KPAD = S  # zero-pad col offset in k tiles

    with tile.TileContext(nc) as tc, ExitStack() as ctx:
        singles = ctx.enter_context(tc.tile_pool(name="singles", bufs=1))
        xtp = ctx.enter_context(tc.tile_pool(name="xt", bufs=BPC))
        maskp = ctx.enter_context(tc.tile_pool(name="mask", bufs=BPC))
        qp = ctx.enter_context(tc.tile_pool(name="qt", bufs=3))
        kp = ctx.enter_context(tc.tile_pool(name="kt", bufs=3))
        vp = ctx.enter_context(tc.tile_pool(name="v", bufs=8 * BPC))
        megap = ctx.enter_context(tc.tile_pool(name="mega", bufs=3))
        ctxp = ctx.enter_context(tc.tile_pool(name="ctx", bufs=BPC))
        ctxTp = ctx.enter_context(tc.tile_pool(name="ctxT", bufs=BPC))
        outp = ctx.enter_context(tc.tile_pool(name="outs", bufs=3))
        rcp = ctx.enter_context(tc.tile_pool(name="rc", bufs=4))
        scp = ctx.enter_context(tc.tile_pool(name="sc", bufs=2, space="PSUM"))
        pcp = ctx.enter_context(tc.tile_pool(name="pc", bufs=2, space="PSUM"))
        mmp = ctx.enter_context(tc.tile_pool(name="mm", bufs=2, space="PSUM"))

        # ---- constants ----
        w_sb = {}

        def load_w(name, d, dt):
            t = singles.tile([128, 4 * E], dt, tag=f"w{name}", name=f"w{name}")
            ov = t[:].rearrange("p (c f) -> p c f", c=4)
            iv = d.rearrange("(c p) f -> p c f", p=128)
            nc.sync.dma_start(out=ov, in_=iv)
            w_sb[name] = t

        bqk_sb = singles.tile([128, 8], F32, tag="bqk")
        nc.sync.dma_start(out=bqk_sb[:], in_=bqk_d[:])
        ident = singles.tile([128, 128], BF16, tag="ident")
        make_identity(nc, ident[:])
        bv_sb = None
        if bv_nonzero:
            bv_sb = singles.tile([128, E], F32, tag="bv")
            nc.sync.dma_start(
                out=bv_sb[:],
                in_=bass.AP(tensor=bv_d.tensor, offset=bv_d.offset,
                            ap=[[0, 128]] + bv_d.ap),
            )
        bo_sb = None
        if bo_nonzero:
            bo_sb = singles.tile([128, E], F32, tag="bo")
            nc.sync.dma_start(
                out=bo_sb[:],
                in_=bass.AP(tensor=bo_d.tensor, offset=bo_d.offset,
                            ap=[[0, 128]] + bo_d.ap),
            )
        warm_in = singles.tile([128, 512], BF16, tag="warm")
        nc.gpsimd.memset(warm_in[:], 0.0)
        warm_ps = mmp.tile([128, 512], F32, tag="mm", name="warmps")
        for _ in range(WARM):
            nc.tensor.matmul(warm_ps[:], lhsT=ident[:], rhs=warm_in[:],
                             start=True, stop=True)

        # ---- pipeline state ----
        xt = {}
        masks = {}
        qk = {}       # (b, 'q'|'k', j) -> fp8 tile
        v_sb = {}     # b -> [8 tiles]
        mega = {}     # (u, hh) -> bf16 tile [128, 8S]
        ctx_sb = {}   # b -> [128, 4096] bf16
        ctxT = {}     # b -> [128, 4096] fp8
        rc_t = {}     # u -> [128, 16] f32
        pd_t = {}     # u -> psum [128, 16]
        drain_ct = {"qk": 0, "out": 0}

        def dma_in_x(b, half=None):
            if b in xt:
                t = xt[b]
            else:
                t = xtp.tile([128, 4 * S], BF16, tag="xt", name=f"xt{b}")
                xt[b] = t
            ov = t[:].rearrange("p (e s) -> p e s", e=4)
            iv = xT_d[b].rearrange("(e p) s -> p e s", p=128)
            if half in (None, 0):
                nc.sync.dma_start(out=ov[:, 0:2], in_=iv[:, 0:2])
            if half in (None, 1):
                nc.sync.dma_start(out=ov[:, 2:4], in_=iv[:, 2:4])

        def dma_in_mask(b, pieces=((0, 4), (4, 8))):
            if b in masks:
                t = masks[b]
            else:
                t = maskp.tile([128, 8 * S], BF16, tag="mask", name=f"mask{b}")
                masks[b] = t
            ov = t[:].rearrange("p (sk sq) -> p sk sq", sk=8)
            iv = maskT_d[b].rearrange("(sk p) sq -> p sk sq", p=128)
            for lo, hi in pieces:
                nc.sync.dma_start(out=ov[:, lo:hi], in_=iv[:, lo:hi])

        def dma_in(b):
            dma_in_x(b)
            dma_in_mask(b)

        def dr_ap(t, prow, nrow, col, stride2, ncol):
            """[nrow part @ prow, 2 @ stride2, ncol] view of tile t."""
            base = t[prow:prow + nrow, col:col + 1]
            return bass.AP(tensor=base.tensor, offset=base.offset,
                           ap=[base.ap[0], [stride2, 2], [1, ncol]])

        def qk_drain(ps, dst, col, sh):
            """PSUM [128,512] -> fp8 q/k slice with bias."""
            i = drain_ct["qk"]
            drain_ct["qk"] += 1
            osl = dst[:, sh * 512:(sh + 1) * 512]
            if i % 8 < (QKDVE + 3) // 4:
                nc.vector.tensor_scalar(osl, ps[:], bqk_sb[:, col:col + 1], None, add)
            else:
                nc.scalar.activation(osl, ps[:], IDENT,
                                     bias=bqk_sb[:, col:col + 1], scale=1.0)

        def proj_qk_steps(b, j):
            steps = []
            qt = qp.tile([128, S], BF16, tag="qt", name=f"q{b}_{j}")
            kt = kp.tile([128, S], BF16, tag="kt", name=f"k{b}_{j}")
            qk[(b, "q", j)] = qt
            qk[(b, "k", j)] = kt
            for ti, (tname, dst) in enumerate((("q", qt), ("k", kt))):
                col = ti * 4 + j
                wname = tname
                for sh in range(2):
                    def mk(tname=wname, dst=dst, col=col, sh=sh):
                        def step():
                            ps = mmp.tile([128, 512], F32, tag="mm",
                                          name=f"pqk{b}{j}{tname}{sh}")
                            for e in range(4):
                                nc.tensor.matmul(
                                    ps[:],
                                    lhsT=w_sb[tname][:, e * E + j * 128: e * E + (j + 1) * 128],
                                    rhs=xt[b][:, e * S + sh * 512: e * S + (sh + 1) * 512],
                                    start=(e == 0), stop=(e == 3),
                                )
                            qk_drain(ps, dst, col, sh)
                        return step
                    steps.append(mk())
            return steps

        def proj_v_steps(b):
            v_sb[b] = [None] * 8
            steps = []

            def mk(s):
                def step():
                    ps = mmp.tile([128, 512], F32, tag="mm", name=f"pv{b}_{s}")
                    for e in range(4):
                        nc.tensor.matmul(
                            ps[:],
                            lhsT=xt[b][:, e * S + s * 128: e * S + (s + 1) * 128],
                            rhs=w_sb["v"][:, e * E:(e + 1) * E],
                            start=(e == 0), stop=(e == 3),
                        )
                    vt = vp.tile([128, 8 * 65], BF16, tag="v", name=f"v{b}_{s}")
                    vv = vt[:].rearrange("p (h c) -> p h c", h=8)
                    pv = ps[:].rearrange("p (h c) -> p h c", h=8)
                    if bv_nonzero:
                        nc.vector.scalar_tensor_tensor(
                            out=vv[:, :, 0:64], in0=pv[:, :, :], scalar=1.0,
                            in1=bv_sb[:].rearrange("p (h c) -> p h c", h=8),
                            op0=mult, op1=add)
                    else:
                        nc.scalar.activation(vv[:, :, 0:64], pv[:, :, :], COPY)
                    nc.gpsimd.memset(vv[:, :, 64:65], 1.0)
                    v_sb[b][s] = vt
                return step

            for s in range(8):
                steps.append(mk(s))
            return steps

        def attn_a(u, bsteps):
            b, j = divmod(u, 4)
            kt = qk[(b, "k", j)]
            qt = qk[(b, "q", j)]
            slot = 0
            nslots = 16
            for hh in range(2):
                mg = megap.tile([128, 8 * S], BF16, tag="mega", name=f"mega{u}_{hh}")
                mega[(u, hh)] = mg
                for sk in range(8):
                    ps = scp.tile([128, S], F32, tag="sc", name=f"sc{u}{hh}{sk}")
                    for sh in range(2):
                        nc.tensor.matmul(
                            ps[:, sh * 512:(sh + 1) * 512],
                            lhsT=kt[hh * 64: hh * 64 + 64, sk * 128:(sk + 1) * 128],
                            rhs=qt[hh * 64: hh * 64 + 64, sh * 512:(sh + 1) * 512],
                            start=True, stop=True,
                        )
                    if sk < DV:
                        nc.vector.scalar_tensor_tensor(
                            out=mg[:, sk * S:(sk + 1) * S],
                            in0=ps[:], scalar=SCH_A,
                            in1=masks[b][:, sk * S:(sk + 1) * S],
                            op0=mult, op1=mult,
                        )
                    else:
                        nc.scalar.activation(mg[:, sk * S:(sk + 1) * S], ps[:], EXP)
                    slots_left = nslots - slot
                    n = (len(bsteps) + slots_left - 1) // slots_left if bsteps else 0
                    for _ in range(n):
                        if bsteps:
                            bsteps.pop(0)()
                    slot += 1
                if DV > 0:
                    mgi = mg[:].bitcast(I16)
                    nc.vector.tensor_scalar(
                        mgi[:, 0:DV * S], mg[:, 0:DV * S], 1.0, SCH_B, mult, add)
                if DV < 8:
                    nc.gpsimd.tensor_tensor(
                        out=mg[:, DV * S:8 * S],
                        in0=mg[:, DV * S:8 * S],
                        in1=masks[b][:, DV * S:8 * S],
                        op=powop,
                    )
            for st in bsteps:
                st()

        def ctx_steps(u):
            b, j = divmod(u, 4)
            steps = []
            if b not in ctx_sb:
                ctx_sb[b] = ctxp.tile([128, 4096], BF16, tag="ctx", name=f"ctx{b}")

            def mk_quarter(q4):
                def step():
                    pc = pcp.tile([128, 260], F32, tag="pc", name=f"pc{u}_{q4}")
                    for i in range(2):
                        sq = q4 * 2 + i
                        for hh in range(2):
                            h = 2 * j + hh
                            mg = mega[(u, hh)]
                            for sk in range(8):
                                nc.tensor.matmul(
                                    pc[:, (i * 2 + hh) * 65:(i * 2 + hh) * 65 + 65],
                                    lhsT=mg[:, sk * S + sq * 128: sk * S + sq * 128 + 128],
                                    rhs=v_sb[b][sk][:, h * 65: h * 65 + 65],
                                    start=(sk == 0), stop=(sk == 7),
                                )
                    rcq = rcp.tile([128, 4], F32, tag="rc", name=f"rc{u}_{q4}")
                    pcv = pc[:].rearrange("p (g c) -> p g c", g=4)
                    nc.vector.reciprocal(rcq[:], pcv[:, :, 64])
                    base = rcq[0:128, 0:1]
                    in1 = bass.AP(tensor=base.tensor, offset=base.offset,
                                  ap=[base.ap[0], [2, 2], [1, 2], [0, 64]])
                    ov = ctx_sb[b][:].rearrange(
                        "p (sq h d) -> p sq h d", sq=8, h=8
                    )[:, q4 * 2:(q4 + 1) * 2, 2 * j:2 * j + 2, :]
                    nc.vector.scalar_tensor_tensor(
                        out=ov, in0=pcv[:, :, 0:64].rearrange("p (i hh) d -> p i hh d", i=2),
                        scalar=1.0, in1=in1, op0=mult, op1=mult)
                return step

            for q4 in range(4):
                steps.append(mk_quarter(q4))
            return steps

        def transpose_steps(u):
            b, j = divmod(u, 4)
            steps = []
            if b not in ctxT:
                ctxT[b] = ctxTp.tile([128, 4096], BF16, tag="ctxT", name=f"ctxT{b}")

            def mk_tr(sq4):
                def step():
                    pt = mmp.tile([128, 512], BF16, tag="mm", name=f"pt{u}_{sq4}")
                    for hh in range(2):
                        h = 2 * j + hh
                        for sqi in range(4):
                            sq = sq4 * 4 + sqi
                            nc.tensor.transpose(
                                out=pt[hh * 64: hh * 64 + 64, sqi * 128:(sqi + 1) * 128],
                                in_=ctx_sb[b][:, sq * 512 + h * 64: sq * 512 + h * 64 + 64],
                                identity=ident[:],
                            )
                    nc.scalar.activation(
                        ctxT[b][:, j * S + sq4 * 512: j * S + (sq4 + 1) * 512],
                        pt[:], COPY)
                return step

            for sq4 in range(2):
                steps.append(mk_tr(sq4))
            return steps

        def outproj_steps(b, half):
            steps = []

            def mk(si):
                def step():
                    s = half * 4 + si
                    po = mmp.tile([128, 512], F32, tag="mm", name=f"po{b}_{s}")
                    for j4 in range(4):
                        nc.tensor.matmul(
                            po[:],
                            lhsT=ctxT[b][:, j4 * S + s * 128: j4 * S + (s + 1) * 128],
                            rhs=w_sb["o"][:, j4 * E:(j4 + 1) * E],
                            start=(j4 == 0), stop=(j4 == 3),
                        )
                    ou = outp.tile([128, 512], F32, tag="outs", name=f"ou{b}_{s}")
                    i = drain_ct["out"]
                    drain_ct["out"] += 1
                    if bo_nonzero:
                        nc.vector.scalar_tensor_tensor(
                            out=ou[:], in0=po[:], scalar=1.0, in1=bo_sb[:],
                            op0=mult, op1=add)
                    elif i % 16 < OUTDVE:
                        nc.vector.tensor_scalar(ou[:], po[:], 1.0, None, mult)
                    else:
                        nc.scalar.activation(ou[:], po[:], COPY)
                    nc.sync.dma_start(
                        out=out_d[b, s * 128:(s + 1) * 128, :], in_=ou[:])
                return step

            for si in range(4):
                steps.append(mk(si))
            return steps

        # ---- emission ----
        load_w("q", wq_d, BF16)
        dma_in_x(0)
        load_w("k", wk_d, BF16)
        dma_in_mask(0, pieces=((0, 1), (1, 2), (2, 4), (4, 8)))
        load_w("v", wv_d, BF16)
        load_w("o", wo_d, BF16)

        for st in proj_qk_steps(0, 0):
            st()
        for st in proj_v_steps(0):
            st()

        for u in range(NU):
            b, j = divmod(u, 4)
            if u == 1 and BPC > 1:
                dma_in(1)
            bsteps = []
            if u + 1 < NU:
                nb, nj = divmod(u + 1, 4)
                bsteps += proj_qk_steps(nb, nj)
            if u == 3 and BPC > 1:
                bsteps += proj_v_steps(1)
            if u >= 1:
                bsteps += ctx_steps(u - 1)
            if u >= 2:
                bsteps += transpose_steps(u - 2)
            if u == 5:
                bsteps += outproj_steps(0, 0)
            if u == 6:
                bsteps += outproj_steps(0, 1)
            attn_a(u, bsteps)

        # tail: interleave ctx(7), transposes(6,7) and outproj(1) so no
        # engine sits behind a long serial chain.
        c7 = ctx_steps(NU - 1)
        t6 = transpose_steps(NU - 2)
        t7 = transpose_steps(NU - 1)
        o0 = outproj_steps(BPC - 1, 0)
        o1 = outproj_steps(BPC - 1, 1)
        order = [c7[0], c7[1], t6[0], c7[2], t6[1], c7[3], t7[0],
                 o0[0], o0[1], t7[1], o0[2], o0[3]] + o1
        for st in order:
            st()

    nc.compile()
    return nc


def _prep(x, adj_matrix, bond_matrix, Wq, bq, Wk, bk, Wv, bv, Wo, bo):
    x = np.asarray(x, np.float32)
    mask = np.asarray(adj_matrix, np.float32) + np.asarray(bond_matrix, np.float32)
    xT = np.ascontiguousarray(x.transpose(0, 2, 1))
    xTb = xT.astype(NPBF16)
    maskT = np.ascontiguousarray(mask.transpose(0, 2, 1)).astype(NPBF16)
    wqT = np.ascontiguousarray(np.asarray(Wq, np.float32).T * SCALE).astype(NPBF16)
    wkT = np.ascontiguousarray(np.asarray(Wk, np.float32).T).astype(NPBF16)
    wvT = np.ascontiguousarray(np.asarray(Wv, np.float32).T).astype(NPBF16)
    woT = np.ascontiguousarray(np.asarray(Wo, np.float32).T).astype(NPBF16)
    bqs = np.asarray(bq, np.float32) * SCALE
    bkf = np.asarray(bk, np.float32)
    bqk = np.concatenate(
        [bqs.reshape(4, 128).T, bkf.reshape(4, 128).T], axis=1).astype(np.float32)
    bqk = np.ascontiguousarray(bqk)
    bvf = np.ascontiguousarray(np.asarray(bv, np.float32))
    bof = np.ascontiguousarray(np.asarray(bo, np.float32))

    in_maps = []
    for c in range(NCORES):
        sl = slice(c * BPC, (c + 1) * BPC)
        in_maps.append({
            "xT": np.ascontiguousarray(xTb[sl]),
            "maskT": np.ascontiguousarray(maskT[sl]),
            "wqT": wqT, "wkT": wkT, "wvT": wvT, "woT": woT,
            "bqk": bqk, "bv": bvf, "bo": bof,
        })
    return in_maps, bool(np.any(bof)), bool(np.any(bvf))


def kernel(x, adj_matrix, bond_matrix, Wq, bq, Wk, bk, Wv, bv, Wo, bo,
           seq_len, _trace=False, _knobs=None):
    in_maps, bo_nonzero, bv_nonzero = _prep(
        x, adj_matrix, bond_matrix, Wq, bq, Wk, bk, Wv, bv, Wo, bo)
    key = ("k", bo_nonzero, bv_nonzero, str(_knobs))
    if key not in _cache:
        _cache[key] = _build(bo_nonzero, bv_nonzero, _knobs)
    nc = _cache[key]
    res = run_bass_kernel_spmd(
        nc, in_maps, core_ids=list(range(NCORES)), trace=_trace)
    out = np.concatenate([r["out"] for r in res.results], axis=0).astype(np.float32)
    if _trace:
        kernel._last_exec_time_ns = res.exec_time_ns
        kernel._last_results = res
    return out


# revision 10
# speedup vs baseline: 1.0148x; 1.0129x over previous
"""AdjMultiHeadAttention Trainium2 kernel, v2.

Sharding: pure data-parallel over batch. B=16 over 8 cores -> 2 batches/core.

Per-core design (driven by the TimelineSim cost model):
  - fp8e4 DoubleRow matmuls (0.5 cyc/row, 2 k-tiles/instr) for the scores,
    v-projection and out-projection; bf16 for qk-projection (accuracy) and
    the ctx matmul (mega must stay bf16).
  - scores are computed transposed (scoresT[sk,sq]) with q,k in fp8 produced
    for free by the projection PSUM drains. The fp8 DR scores matmul uses a
    zero-padded second k-tile in the stationary operand (contraction is only
    d=64) -- cost is halved vs bf16 regardless.
  - The elementwise wall (mask-multiply + exp on 16.8M f32 PSUM elements per
    core) is split three ways:
      * DV of 8 sk-tiles per (u,hh): DVE drains with the mask fused
        (scalar_tensor_tensor, x184.66496 Schraudolph prescale), then one
        cheap 4x-mode tensor_scalar (+16248.577 -> int16) computes exp via
        the Schraudolph bf16-bit trick.
      * the rest: ACT drains with func=Exp fused (exact exp of the raw
        scores), then gpsimd computes E^mask with tensor_tensor(op=pow)
        (exp(s*m) == exp(s)^m), one big op per (u,hh).
  - softmax denominators ride along as 1-wide ones-column matmuls into a tiny
    PSUM tile (PE cost ~0 in the model); normalization is fused into the
    PSUM->SBUF ctx drain via a per-partition-broadcast reciprocal.
  - ctx is transposed 128x64-block-wise on the PE (bf16), drained to fp8 for
    the DR out-projection.
Emission is software-pipelined over units u=(batch, head-pair) with a step
backlog popped between score tiles, as in v1.
"""

import os
import sys

sys.path.insert(0, "/opt/trn_rl_repo")

from contextlib import ExitStack

import ml_dtypes
import numpy as np

import concourse.bass as bass
import concourse.tile as tile
from concourse import bacc, mybir
from concourse.bass_utils import run_bass_kernel_spmd
from concourse.masks import make_identity

B, S, E, H, D = 16, 1024, 512, 8, 64
NCORES = 8
BPC = B // NCORES
SCALE = D**-0.5
BF16 = mybir.dt.bfloat16
F32 = mybir.dt.float32
F8 = mybir.dt.float8e4
I16 = mybir.dt.int16
NPBF16 = ml_dtypes.bfloat16
NPF8 = ml_dtypes.float8_e4m3

# Schraudolph constants for bf16 exp-by-bits: int16bits(bf16(e^x)) ~=
# round(x*184.66496 + 16248.577)
SCH_A = 184.66496
SCH_B = 16248.577

_cache = {}

NU = BPC * 4


def _build(bo_nonzero: bool, bv_nonzero: bool = False, knobs=None):
    knobs = knobs or {}
    WARM = int(knobs.get("warm", os.environ.get("K_WARM", 8)))
    DV = int(knobs.get("dv", os.environ.get("K_DV", 4)))       # DVE-drained sk per (u,hh)
    QKDVE = int(knobs.get("qkdve", os.environ.get("K_QKDVE", 4)))  # of 32 qk drains on DVE
    OUTDVE = int(knobs.get("outdve", os.environ.get("K_OUTDVE", 0)))  # of 16 out drains on DVE
    assert 0 <= DV <= 8

    nc = bacc.Bacc("TRN2", target_bir_lowering=False, debug=False, num_devices=NCORES)

    xT_d = nc.dram_tensor("xT", [BPC, E, S], BF16, kind="ExternalInput").ap()
    maskT_d = nc.dram_tensor("maskT", [BPC, S, S], BF16, kind="ExternalInput").ap()
    wq_d = nc.dram_tensor("wqT", [E, E], BF16, kind="ExternalInput").ap()
    wk_d = nc.dram_tensor("wkT", [E, E], BF16, kind="ExternalInput").ap()
    wv_d = nc.dram_tensor("wvT", [E, E], BF16, kind="ExternalInput").ap()
    wo_d = nc.dram_tensor("woT", [E, E], BF16, kind="ExternalInput").ap()
    bqk_d = nc.dram_tensor("bqk", [128, 8], F32, kind="ExternalInput").ap()
    bv_d = nc.dram_tensor("bv", [E], F32, kind="ExternalInput").ap()
    bo_d = nc.dram_tensor("bo", [E], F32, kind="ExternalInput").ap()
    out_d = nc.dram_tensor("out", [BPC, S, E], F32, kind="ExternalOutput").ap()

    mult = mybir.AluOpType.mult
    add = mybir.AluOpType.add
    powop = mybir.AluOpType.pow
    EXP = mybir.ActivationFunctionType.Exp
    IDENT = mybir.ActivationFunctionType.Identity
    COPY = mybir.ActivationFunctionType.Copy
    DRMODE = mybir.MatmulPerfMode.DoubleRow

    KPAD = S  # zero-pad col offset in k tiles

    with tile.TileContext(nc) as tc, ExitStack() as ctx:
        singles = ctx.enter_context(tc.tile_pool(name="singles", bufs=1))
        xtp = ctx.enter_context(tc.tile_pool(name="xt", bufs=BPC))
        maskp = ctx.enter_context(tc.tile_pool(name="mask", bufs=BPC))
        qp = ctx.enter_context(tc.tile_pool(name="qt", bufs=3))
        kp = ctx.enter_context(tc.tile_pool(name="kt", bufs=3))
        vp = ctx.enter_context(tc.tile_pool(name="v", bufs=8 * BPC))
        megap = ctx.enter_context(tc.tile_pool(name="mega", bufs=4))
        ctxp = ctx.enter_context(tc.tile_pool(name="ctx", bufs=BPC))
        ctxTp = ctx.enter_context(tc.tile_pool(name="ctxT", bufs=BPC))
        outp = ctx.enter_context(tc.tile_pool(name="outs", bufs=3))
        rcp = ctx.enter_context(tc.tile_pool(name="rc", bufs=4))
        scp = ctx.enter_context(tc.tile_pool(name="sc", bufs=2, space="PSUM"))
        pcp = ctx.enter_context(tc.tile_pool(name="pc", bufs=2, space="PSUM"))
        mmp = ctx.enter_context(tc.tile_pool(name="mm", bufs=2, space="PSUM"))

        # ---- constants ----
        w_sb = {}

        def load_w(name, d, dt):
            t = singles.tile([128, 4 * E], dt, tag=f"w{name}", name=f"w{name}")
            ov = t[:].rearrange("p (c f) -> p c f", c=4)
            iv = d.rearrange("(c p) f -> p c f", p=128)
            nc.sync.dma_start(out=ov, in_=iv)
            w_sb[name] = t

        bqk_sb = singles.tile([128, 8], F32, tag="bqk")
        nc.sync.dma_start(out=bqk_sb[:], in_=bqk_d[:])
        ident = singles.tile([128, 128], BF16, tag="ident")
        make_identity(nc, ident[:])
        bv_sb = None
        if bv_nonzero:
            bv_sb = singles.tile([128, E], F32, tag="bv")
            nc.sync.dma_start(
                out=bv_sb[:],
                in_=bass.AP(tensor=bv_d.tensor, offset=bv_d.offset,
                            ap=[[0, 128]] + bv_d.ap),
            )
        bo_sb = None
        if bo_nonzero:
            bo_sb = singles.tile([128, E], F32, tag="bo")
            nc.sync.dma_start(
                out=bo_sb[:],
                in_=bass.AP(tensor=bo_d.tensor, offset=bo_d.offset,
                            ap=[[0, 128]] + bo_d.ap),
            )
        warm_in = singles.tile([128, 512], BF16, tag="warm")
        nc.gpsimd.memset(warm_in[:], 0.0)
        warm_ps = mmp.tile([128, 512], F32, tag="mm", name="warmps")
        for _ in range(WARM):
            nc.tensor.matmul(warm_ps[:], lhsT=ident[:], rhs=warm_in[:],
                             start=True, stop=True)

        # ---- pipeline state ----
        xt = {}
        masks = {}
        qk = {}       # (b, 'q'|'k', j) -> fp8 tile
        v_sb = {}     # b -> [8 tiles]
        mega = {}     # (u, hh) -> bf16 tile [128, 8S]
        ctx_sb = {}   # b -> [128, 4096] bf16
        ctxT = {}     # b -> [128, 4096] fp8
        rc_t = {}     # u -> [128, 16] f32
        pd_t = {}     # u -> psum [128, 16]
        drain_ct = {"qk": 0, "out": 0}

        def dma_in_x(b, half=None):
            if b in xt:
                t = xt[b]
            else:
                t = xtp.tile([128, 4 * S], BF16, tag="xt", name=f"xt{b}")
                xt[b] = t
            ov = t[:].rearrange("p (e s) -> p e s", e=4)
            iv = xT_d[b].rearrange("(e p) s -> p e s", p=128)
            if half in (None, 0):
                nc.sync.dma_start(out=ov[:, 0:2], in_=iv[:, 0:2])
            if half in (None, 1):
                nc.sync.dma_start(out=ov[:, 2:4], in_=iv[:, 2:4])

        def dma_in_mask(b, pieces=((0, 4), (4, 8))):
            if b in masks:
                t = masks[b]
            else:
                t = maskp.tile([128, 8 * S], BF16, tag="mask", name=f"mask{b}")
                masks[b] = t
            ov = t[:].rearrange("p (sk sq) -> p sk sq", sk=8)
            iv = maskT_d[b].rearrange("(sk p) sq -> p sk sq", p=128)
            for lo, hi in pieces:
                nc.sync.dma_start(out=ov[:, lo:hi], in_=iv[:, lo:hi])

        def dma_in(b):
            dma_in_x(b)
            dma_in_mask(b)

        def dr_ap(t, prow, nrow, col, stride2, ncol):
            """[nrow part @ prow, 2 @ stride2, ncol] view of tile t."""
            base = t[prow:prow + nrow, col:col + 1]
            return bass.AP(tensor=base.tensor, offset=base.offset,
                           ap=[base.ap[0], [stride2, 2], [1, ncol]])

        def qk_drain(ps, dst, col, sh):
            """PSUM [128,512] -> fp8 q/k slice with bias."""
            i = drain_ct["qk"]
            drain_ct["qk"] += 1
            osl = dst[:, sh * 512:(sh + 1) * 512]
            if i % 8 < (QKDVE + 3) // 4:
                nc.vector.tensor_scalar(osl, ps[:], bqk_sb[:, col:col + 1], None, add)
            else:
                nc.scalar.activation(osl, ps[:], IDENT,
                                     bias=bqk_sb[:, col:col + 1], scale=1.0)

        def proj_qk_steps(b, j):
            steps = []
            qt = qp.tile([128, S], BF16, tag="qt", name=f"q{b}_{j}")
            kt = kp.tile([128, S], BF16, tag="kt", name=f"k{b}_{j}")
            qk[(b, "q", j)] = qt
            qk[(b, "k", j)] = kt
            for ti, (tname, dst) in enumerate((("q", qt), ("k", kt))):
                col = ti * 4 + j
                wname = tname
                for sh in range(2):
                    def mk(tname=wname, dst=dst, col=col, sh=sh):
                        def step():
                            ps = mmp.tile([128, 512], F32, tag="mm",
                                          name=f"pqk{b}{j}{tname}{sh}")
                            for e in range(4):
                                nc.tensor.matmul(
                                    ps[:],
                                    lhsT=w_sb[tname][:, e * E + j * 128: e * E + (j + 1) * 128],
                                    rhs=xt[b][:, e * S + sh * 512: e * S + (sh + 1) * 512],
                                    start=(e == 0), stop=(e == 3),
                                )
                            qk_drain(ps, dst, col, sh)
                        return step
                    steps.append(mk())
            return steps

        def proj_v_steps(b):
            v_sb[b] = [None] * 8
            steps = []

            def mk(s):
                def step():
                    ps = mmp.tile([128, 512], F32, tag="mm", name=f"pv{b}_{s}")
                    for e in range(4):
                        nc.tensor.matmul(
                            ps[:],
                            lhsT=xt[b][:, e * S + s * 128: e * S + (s + 1) * 128],
                            rhs=w_sb["v"][:, e * E:(e + 1) * E],
                            start=(e == 0), stop=(e == 3),
                        )
                    vt = vp.tile([128, 8 * 65], BF16, tag="v", name=f"v{b}_{s}")
                    vv = vt[:].rearrange("p (h c) -> p h c", h=8)
                    pv = ps[:].rearrange("p (h c) -> p h c", h=8)
                    if bv_nonzero:
                        nc.vector.scalar_tensor_tensor(
                            out=vv[:, :, 0:64], in0=pv[:, :, :], scalar=1.0,
                            in1=bv_sb[:].rearrange("p (h c) -> p h c", h=8),
                            op0=mult, op1=add)
                    else:
                        nc.scalar.activation(vv[:, :, 0:64], pv[:, :, :], COPY)
                    nc.gpsimd.memset(vv[:, :, 64:65], 1.0)
                    v_sb[b][s] = vt
                return step

            for s in range(8):
                steps.append(mk(s))
            return steps

        def attn_a(u, bsteps):
            b, j = divmod(u, 4)
            kt = qk[(b, "k", j)]
            qt = qk[(b, "q", j)]
            slot = 0
            nslots = 16
            for hh in range(2):
                mg = megap.tile([128, 8 * S], BF16, tag="mega", name=f"mega{u}_{hh}")
                mega[(u, hh)] = mg
                for sk in range(8):
                    ps = scp.tile([128, S], F32, tag="sc", name=f"sc{u}{hh}{sk}")
                    for sh in range(2):
                        nc.tensor.matmul(
                            ps[:, sh * 512:(sh + 1) * 512],
                            lhsT=kt[hh * 64: hh * 64 + 64, sk * 128:(sk + 1) * 128],
                            rhs=qt[hh * 64: hh * 64 + 64, sh * 512:(sh + 1) * 512],
                            start=True, stop=True,
                        )
                    if sk < DV:
                        nc.vector.scalar_tensor_tensor(
                            out=mg[:, sk * S:(sk + 1) * S],
                            in0=ps[:], scalar=SCH_A,
                            in1=masks[b][:, sk * S:(sk + 1) * S],
                            op0=mult, op1=mult,
                        )
                        if sk == DV - 1 or sk % 2 == 1:
                            lo = (sk // 2) * 2
                            hi = min(sk + 1, DV)
                            mgi = mg[:].bitcast(I16)
                            nc.vector.tensor_scalar(
                                mgi[:, lo * S:hi * S], mg[:, lo * S:hi * S],
                                1.0, SCH_B, mult, add)
                    else:
                        nc.scalar.activation(mg[:, sk * S:(sk + 1) * S], ps[:], EXP)
                        if sk == 7 or (sk - DV) % 2 == 1:
                            lo = max(DV, sk - ((sk - DV) % 2))
                            hi = sk + 1
                            nc.gpsimd.tensor_tensor(
                                out=mg[:, lo * S:hi * S],
                                in0=mg[:, lo * S:hi * S],
                                in1=masks[b][:, lo * S:hi * S],
                                op=powop,
                            )
                    slots_left = nslots - slot
                    n = (len(bsteps) + slots_left - 1) // slots_left if bsteps else 0
                    for _ in range(n):
                        if bsteps:
                            bsteps.pop(0)()
                    slot += 1
            for st in bsteps:
                st()

        def ctx_steps(u):
            b, j = divmod(u, 4)
            steps = []
            if b not in ctx_sb:
                ctx_sb[b] = ctxp.tile([128, 4096], BF16, tag="ctx", name=f"ctx{b}")

            pcs = {}

            def mk_chains(q4):
                def step():
                    pc = pcp.tile([128, 260], F32, tag="pc", name=f"pc{u}_{q4}")
                    pcs[q4] = pc
                    for i in range(2):
                        sq = q4 * 2 + i
                        for hh in range(2):
                            h = 2 * j + hh
                            mg = mega[(u, hh)]
                            for sk in range(8):
                                nc.tensor.matmul(
                                    pc[:, (i * 2 + hh) * 65:(i * 2 + hh) * 65 + 65],
                                    lhsT=mg[:, sk * S + sq * 128: sk * S + sq * 128 + 128],
                                    rhs=v_sb[b][sk][:, h * 65: h * 65 + 65],
                                    start=(sk == 0), stop=(sk == 7),
                                )
                return step

            def mk_norm(q4):
                def step():
                    pc = pcs[q4]
                    rcq = rcp.tile([128, 4], F32, tag="rc", name=f"rc{u}_{q4}")
                    pcv = pc[:].rearrange("p (g c) -> p g c", g=4)
                    nc.vector.reciprocal(rcq[:], pcv[:, :, 64])
                    base = rcq[0:128, 0:1]
                    in1 = bass.AP(tensor=base.tensor, offset=base.offset,
                                  ap=[base.ap[0], [2, 2], [1, 2], [0, 64]])
                    ov = ctx_sb[b][:].rearrange(
                        "p (sq h d) -> p sq h d", sq=8, h=8
                    )[:, q4 * 2:(q4 + 1) * 2, 2 * j:2 * j + 2, :]
                    nc.vector.scalar_tensor_tensor(
                        out=ov, in0=pcv[:, :, 0:64].rearrange("p (i hh) d -> p i hh d", i=2),
                        scalar=1.0, in1=in1, op0=mult, op1=mult)
                return step

            c = [mk_chains(q4) for q4 in range(4)]
            n = [mk_norm(q4) for q4 in range(4)]
            steps += [c[0], c[1], n[0], c[2], n[1], c[3], n[2], n[3]]
            return steps

        def transpose_steps(u):
            b, j = divmod(u, 4)
            steps = []
            if b not in ctxT:
                ctxT[b] = ctxTp.tile([128, 4096], BF16, tag="ctxT", name=f"ctxT{b}")

            def mk_tr(sq4):
                def step():
                    for sqi in range(4):
                        sq = sq4 * 4 + sqi
                        nc.scalar.dma_start_transpose(
                            out=ctxT[b][:, j * S + sq * 128:(j * S + (sq + 1) * 128)],
                            in_=ctx_sb[b][:, sq * 512 + j * 128: sq * 512 + (j + 1) * 128],
                        )
                return step

            for sq4 in range(2):
                steps.append(mk_tr(sq4))
            return steps

        def outproj_steps(b, half):
            steps = []

            def mk(si):
                def step():
                    s = half * 4 + si
                    po = mmp.tile([128, 512], F32, tag="mm", name=f"po{b}_{s}")
                    for j4 in range(4):
                        nc.tensor.matmul(
                            po[:],
                            lhsT=ctxT[b][:, j4 * S + s * 128: j4 * S + (s + 1) * 128],
                            rhs=w_sb["o"][:, j4 * E:(j4 + 1) * E],
                            start=(j4 == 0), stop=(j4 == 3),
                        )
                    ou = outp.tile([128, 512], F32, tag="outs", name=f"ou{b}_{s}")
                    i = drain_ct["out"]
                    drain_ct["out"] += 1
                    if bo_nonzero:
                        nc.vector.scalar_tensor_tensor(
                            out=ou[:], in0=po[:], scalar=1.0, in1=bo_sb[:],
                            op0=mult, op1=add)
                    elif i % 16 < OUTDVE:
                        nc.vector.tensor_scalar(ou[:], po[:], 1.0, None, mult)
                    else:
                        nc.scalar.activation(ou[:], po[:], COPY)
                    nc.scalar.dma_start(
                        out=out_d[b, s * 128:(s + 1) * 128, :], in_=ou[:])
                return step

            for si in range(4):
                steps.append(mk(si))
            return steps

        # ---- emission ----
        load_w("q", wq_d, BF16)
        dma_in_x(0)
        load_w("k", wk_d, BF16)
        dma_in_mask(0, pieces=((0, 1), (1, 2), (2, 4), (4, 8)))
        load_w("v", wv_d, BF16)
        load_w("o", wo_d, BF16)

        for st in proj_qk_steps(0, 0):
            st()
        for st in proj_v_steps(0):
            st()

        for u in range(NU):
            b, j = divmod(u, 4)
            if u == 1 and BPC > 1:
                dma_in(1)
            bsteps = []
            if u + 1 < NU:
                nb, nj = divmod(u + 1, 4)
                bsteps += proj_qk_steps(nb, nj)
            if u == 3 and BPC > 1:
                bsteps += proj_v_steps(1)
            if u >= 1:
                bsteps += ctx_steps(u - 1)
            if u >= 2:
                bsteps += transpose_steps(u - 2)
            if u == 5:
                bsteps += outproj_steps(0, 0)
            if u == 6:
                bsteps += outproj_steps(0, 1)
            attn_a(u, bsteps)

        # tail: interleave ctx(7), transposes(6,7) and outproj(1) so no
        # engine sits behind a long serial chain.
        c7 = ctx_steps(NU - 1)  # [c0,c1,n0,c2,n1,c3,n2,n3]
        t6 = transpose_steps(NU - 2)
        t7 = transpose_steps(NU - 1)
        o0 = outproj_steps(BPC - 1, 0)
        o1 = outproj_steps(BPC - 1, 1)
        order = [c7[0], c7[1], t6[0], c7[2], c7[3], t6[1], c7[4], c7[5],
                 c7[6], c7[7], t7[0], o0[0], o0[1], t7[1], o0[2], o0[3]] + o1
        for st in order:
            st()

    nc.compile()
    return nc


def _prep(x, adj_matrix, bond_matrix, Wq, bq, Wk, bk, Wv, bv, Wo, bo):
    x = np.asarray(x, np.float32)
    mask = np.asarray(adj_matrix, np.float32) + np.asarray(bond_matrix, np.float32)
    xT = np.ascontiguousarray(x.transpose(0, 2, 1))
    xTb = xT.astype(NPBF16)
    maskT = np.ascontiguousarray(mask.transpose(0, 2, 1)).astype(NPBF16)
    wqT = np.ascontiguousarray(np.asarray(Wq, np.float32).T * SCALE).astype(NPBF16)
    wkT = np.ascontiguousarray(np.asarray(Wk, np.float32).T).astype(NPBF16)
    wvT = np.ascontiguousarray(np.asarray(Wv, np.float32).T).astype(NPBF16)
    woT = np.ascontiguousarray(np.asarray(Wo, np.float32).T).astype(NPBF16)
    bqs = np.asarray(bq, np.float32) * SCALE
    bkf = np.asarray(bk, np.float32)
    bqk = np.concatenate(
        [bqs.reshape(4, 128).T, bkf.reshape(4, 128).T], axis=1).astype(np.float32)
    bqk = np.ascontiguousarray(bqk)
    bvf = np.ascontiguousarray(np.asarray(bv, np.float32))
    bof = np.ascontiguousarray(np.asarray(bo, np.float32))

    in_maps = []
    for c in range(NCORES):
        sl = slice(c * BPC, (c + 1) * BPC)
        in_maps.append({
            "xT": np.ascontiguousarray(xTb[sl]),
            "maskT": np.ascontiguousarray(maskT[sl]),
            "wqT": wqT, "wkT": wkT, "wvT": wvT, "woT": woT,
            "bqk": bqk, "bv": bvf, "bo": bof,
        })
    return in_maps, bool(np.any(bof)), bool(np.any(bvf))


def kernel(x, adj_matrix, bond_matrix, Wq, bq, Wk, bk, Wv, bv, Wo, bo,
           seq_len, _trace=False, _knobs=None):
    in_maps, bo_nonzero, bv_nonzero = _prep(
        x, adj_matrix, bond_matrix, Wq, bq, Wk, bk, Wv, bv, Wo, bo)
    key = ("k", bo_nonzero, bv_nonzero, str(_knobs))
    if key not in _cache:
        _cache[key] = _build(bo_nonzero, bv_nonzero, _knobs)
    nc = _cache[key]
    res = run_bass_kernel_spmd(
        nc, in_maps, core_ids=list(range(NCORES)), trace=_trace)
    out = np.concatenate([r["out"] for r in res.results], axis=0).astype(np.float32)
    if _trace:
        kernel._last_exec_time_ns = res.exec_time_ns
        kernel._last_results = res
    return out


# revision 11
# speedup vs baseline: 1.0226x; 1.0076x over previous
"""AdjMultiHeadAttention Trainium2 kernel, v2.

Sharding: pure data-parallel over batch. B=16 over 8 cores -> 2 batches/core.

Per-core design (driven by the TimelineSim cost model):
  - fp8e4 DoubleRow matmuls (0.5 cyc/row, 2 k-tiles/instr) for the scores,
    v-projection and out-projection; bf16 for qk-projection (accuracy) and
    the ctx matmul (mega must stay bf16).
  - scores are computed transposed (scoresT[sk,sq]) with q,k in fp8 produced
    for free by the projection PSUM drains. The fp8 DR scores matmul uses a
    zero-padded second k-tile in the stationary operand (contraction is only
    d=64) -- cost is halved vs bf16 regardless.
  - The elementwise wall (mask-multiply + exp on 16.8M f32 PSUM elements per
    core) is split three ways:
      * DV of 8 sk-tiles per (u,hh): DVE drains with the mask fused
        (scalar_tensor_tensor, x184.66496 Schraudolph prescale), then one
        cheap 4x-mode tensor_scalar (+16248.577 -> int16) computes exp via
        the Schraudolph bf16-bit trick.
      * the rest: ACT drains with func=Exp fused (exact exp of the raw
        scores), then gpsimd computes E^mask with tensor_tensor(op=pow)
        (exp(s*m) == exp(s)^m), one big op per (u,hh).
  - softmax denominators ride along as 1-wide ones-column matmuls into a tiny
    PSUM tile (PE cost ~0 in the model); normalization is fused into the
    PSUM->SBUF ctx drain via a per-partition-broadcast reciprocal.
  - ctx is transposed 128x64-block-wise on the PE (bf16), drained to fp8 for
    the DR out-projection.
Emission is software-pipelined over units u=(batch, head-pair) with a step
backlog popped between score tiles, as in v1.
"""

import os
import sys

sys.path.insert(0, "/opt/trn_rl_repo")

from contextlib import ExitStack

import ml_dtypes
import numpy as np

import concourse.bass as bass
import concourse.tile as tile
from concourse import bacc, mybir
from concourse.bass_utils import run_bass_kernel_spmd
from concourse.masks import make_identity

B, S, E, H, D = 16, 1024, 512, 8, 64
NCORES = 8
BPC = B // NCORES
SCALE = D**-0.5
BF16 = mybir.dt.bfloat16
F32 = mybir.dt.float32
F8 = mybir.dt.float8e4
I16 = mybir.dt.int16
NPBF16 = ml_dtypes.bfloat16
NPF8 = ml_dtypes.float8_e4m3

# Schraudolph constants for bf16 exp-by-bits: int16bits(bf16(e^x)) ~=
# round(x*184.66496 + 16248.577)
SCH_A = 184.66496
SCH_B = 16248.577

_cache = {}

NU = BPC * 4


def _build(bo_nonzero: bool, bv_nonzero: bool = False, knobs=None):
    knobs = knobs or {}
    WARM = int(knobs.get("warm", os.environ.get("K_WARM", 8)))
    DV = int(knobs.get("dv", os.environ.get("K_DV", 4)))       # DVE-drained sk per (u,hh)
    QKDVE = int(knobs.get("qkdve", os.environ.get("K_QKDVE", 4)))  # of 32 qk drains on DVE
    OUTDVE = int(knobs.get("outdve", os.environ.get("K_OUTDVE", 0)))  # of 16 out drains on DVE
    assert 0 <= DV <= 8

    nc = bacc.Bacc("TRN2", target_bir_lowering=False, debug=False, num_devices=NCORES)

    xT_d = nc.dram_tensor("xT", [BPC, E, S], BF16, kind="ExternalInput").ap()
    maskT_d = nc.dram_tensor("maskT", [BPC, S, S], BF16, kind="ExternalInput").ap()
    wq_d = nc.dram_tensor("wqT", [E, E], BF16, kind="ExternalInput").ap()
    wk_d = nc.dram_tensor("wkT", [E, E], BF16, kind="ExternalInput").ap()
    wv_d = nc.dram_tensor("wvT", [E, E], BF16, kind="ExternalInput").ap()
    wo_d = nc.dram_tensor("woT", [E, E], BF16, kind="ExternalInput").ap()
    bqk_d = nc.dram_tensor("bqk", [128, 8], F32, kind="ExternalInput").ap()
    bv_d = nc.dram_tensor("bv", [E], F32, kind="ExternalInput").ap()
    bo_d = nc.dram_tensor("bo", [E], F32, kind="ExternalInput").ap()
    out_d = nc.dram_tensor("out", [BPC, S, E], F32, kind="ExternalOutput").ap()

    mult = mybir.AluOpType.mult
    add = mybir.AluOpType.add
    powop = mybir.AluOpType.pow
    EXP = mybir.ActivationFunctionType.Exp
    IDENT = mybir.ActivationFunctionType.Identity
    COPY = mybir.ActivationFunctionType.Copy
    DRMODE = mybir.MatmulPerfMode.DoubleRow

    KPAD = S  # zero-pad col offset in k tiles

    with tile.TileContext(nc) as tc, ExitStack() as ctx:
        singles = ctx.enter_context(tc.tile_pool(name="singles", bufs=1))
        xtp = ctx.enter_context(tc.tile_pool(name="xt", bufs=BPC))
        maskp = ctx.enter_context(tc.tile_pool(name="mask", bufs=BPC))
        qp = ctx.enter_context(tc.tile_pool(name="qt", bufs=3))
        kp = ctx.enter_context(tc.tile_pool(name="kt", bufs=3))
        vp = ctx.enter_context(tc.tile_pool(name="v", bufs=8 * BPC))
        megap = ctx.enter_context(tc.tile_pool(name="mega", bufs=4))
        ctxp = ctx.enter_context(tc.tile_pool(name="ctx", bufs=BPC))
        ctxTp = ctx.enter_context(tc.tile_pool(name="ctxT", bufs=BPC))
        outp = ctx.enter_context(tc.tile_pool(name="outs", bufs=3))
        rcp = ctx.enter_context(tc.tile_pool(name="rc", bufs=4))
        scp = ctx.enter_context(tc.tile_pool(name="sc", bufs=2, space="PSUM"))
        pcp = ctx.enter_context(tc.tile_pool(name="pc", bufs=2, space="PSUM"))
        mmp = ctx.enter_context(tc.tile_pool(name="mm", bufs=2, space="PSUM"))

        # ---- constants ----
        w_sb = {}

        def load_w(name, d, dt):
            t = singles.tile([128, 4 * E], dt, tag=f"w{name}", name=f"w{name}")
            ov = t[:].rearrange("p (c f) -> p c f", c=4)
            iv = d.rearrange("(c p) f -> p c f", p=128)
            nc.sync.dma_start(out=ov, in_=iv)
            w_sb[name] = t

        bqk_sb = singles.tile([128, 8], F32, tag="bqk")
        nc.sync.dma_start(out=bqk_sb[:], in_=bqk_d[:])
        ident = singles.tile([128, 128], BF16, tag="ident")
        make_identity(nc, ident[:])
        bv_sb = None
        if bv_nonzero:
            bv_sb = singles.tile([128, E], F32, tag="bv")
            nc.sync.dma_start(
                out=bv_sb[:],
                in_=bass.AP(tensor=bv_d.tensor, offset=bv_d.offset,
                            ap=[[0, 128]] + bv_d.ap),
            )
        bo_sb = None
        if bo_nonzero:
            bo_sb = singles.tile([128, E], F32, tag="bo")
            nc.sync.dma_start(
                out=bo_sb[:],
                in_=bass.AP(tensor=bo_d.tensor, offset=bo_d.offset,
                            ap=[[0, 128]] + bo_d.ap),
            )
        warm_in = singles.tile([128, 512], BF16, tag="warm")
        nc.gpsimd.memset(warm_in[:], 0.0)
        warm_ps = mmp.tile([128, 512], F32, tag="mm", name="warmps")
        for _ in range(WARM):
            nc.tensor.matmul(warm_ps[:], lhsT=ident[:], rhs=warm_in[:],
                             start=True, stop=True)

        # ---- pipeline state ----
        xt = {}
        masks = {}
        qk = {}       # (b, 'q'|'k', j) -> fp8 tile
        v_sb = {}     # b -> [8 tiles]
        mega = {}     # (u, hh) -> bf16 tile [128, 8S]
        ctx_sb = {}   # b -> [128, 4096] bf16
        ctxT = {}     # b -> [128, 4096] fp8
        rc_t = {}     # u -> [128, 16] f32
        pd_t = {}     # u -> psum [128, 16]
        drain_ct = {"qk": 0, "out": 0}

        def dma_in_x(b, half=None):
            if b in xt:
                t = xt[b]
            else:
                t = xtp.tile([128, 4 * S], BF16, tag="xt", name=f"xt{b}")
                xt[b] = t
            ov = t[:].rearrange("p (e s) -> p e s", e=4)
            iv = xT_d[b].rearrange("(e p) s -> p e s", p=128)
            if half in (None, 0):
                nc.sync.dma_start(out=ov[:, 0:2], in_=iv[:, 0:2])
            if half in (None, 1):
                nc.sync.dma_start(out=ov[:, 2:4], in_=iv[:, 2:4])

        def dma_in_mask(b, pieces=((0, 4), (4, 8))):
            if b in masks:
                t = masks[b]
            else:
                t = maskp.tile([128, 8 * S], BF16, tag="mask", name=f"mask{b}")
                masks[b] = t
            ov = t[:].rearrange("p (sk sq) -> p sk sq", sk=8)
            iv = maskT_d[b].rearrange("(sk p) sq -> p sk sq", p=128)
            for lo, hi in pieces:
                nc.sync.dma_start(out=ov[:, lo:hi], in_=iv[:, lo:hi])

        def dma_in(b):
            dma_in_x(b)
            dma_in_mask(b)

        def dr_ap(t, prow, nrow, col, stride2, ncol):
            """[nrow part @ prow, 2 @ stride2, ncol] view of tile t."""
            base = t[prow:prow + nrow, col:col + 1]
            return bass.AP(tensor=base.tensor, offset=base.offset,
                           ap=[base.ap[0], [stride2, 2], [1, ncol]])

        def qk_drain(ps, dst, col, sh):
            """PSUM [128,512] -> fp8 q/k slice with bias."""
            i = drain_ct["qk"]
            drain_ct["qk"] += 1
            osl = dst[:, sh * 512:(sh + 1) * 512]
            if i % 8 < (QKDVE + 3) // 4:
                nc.vector.tensor_scalar(osl, ps[:], bqk_sb[:, col:col + 1], None, add)
            else:
                nc.scalar.activation(osl, ps[:], IDENT,
                                     bias=bqk_sb[:, col:col + 1], scale=1.0)

        def proj_qk_steps(b, j):
            steps = []
            qt = qp.tile([128, S], BF16, tag="qt", name=f"q{b}_{j}")
            kt = kp.tile([128, S], BF16, tag="kt", name=f"k{b}_{j}")
            qk[(b, "q", j)] = qt
            qk[(b, "k", j)] = kt
            for ti, (tname, dst) in enumerate((("q", qt), ("k", kt))):
                col = ti * 4 + j
                wname = tname
                for sh in range(2):
                    def mk(tname=wname, dst=dst, col=col, sh=sh):
                        def step():
                            ps = mmp.tile([128, 512], F32, tag="mm",
                                          name=f"pqk{b}{j}{tname}{sh}")
                            for e in range(4):
                                nc.tensor.matmul(
                                    ps[:],
                                    lhsT=w_sb[tname][:, e * E + j * 128: e * E + (j + 1) * 128],
                                    rhs=xt[b][:, e * S + sh * 512: e * S + (sh + 1) * 512],
                                    start=(e == 0), stop=(e == 3),
                                )
                            qk_drain(ps, dst, col, sh)
                        return step
                    steps.append(mk())
            return steps

        def proj_v_steps(b):
            v_sb[b] = [None] * 8
            steps = []

            def mk(s):
                def step():
                    ps = mmp.tile([128, 512], F32, tag="mm", name=f"pv{b}_{s}")
                    for e in range(4):
                        nc.tensor.matmul(
                            ps[:],
                            lhsT=xt[b][:, e * S + s * 128: e * S + (s + 1) * 128],
                            rhs=w_sb["v"][:, e * E:(e + 1) * E],
                            start=(e == 0), stop=(e == 3),
                        )
                    vt = vp.tile([128, 8 * 65], BF16, tag="v", name=f"v{b}_{s}")
                    vv = vt[:].rearrange("p (h c) -> p h c", h=8)
                    pv = ps[:].rearrange("p (h c) -> p h c", h=8)
                    if bv_nonzero:
                        nc.vector.scalar_tensor_tensor(
                            out=vv[:, :, 0:64], in0=pv[:, :, :], scalar=1.0,
                            in1=bv_sb[:].rearrange("p (h c) -> p h c", h=8),
                            op0=mult, op1=add)
                    else:
                        nc.scalar.activation(vv[:, :, 0:64], pv[:, :, :], COPY)
                    nc.gpsimd.memset(vv[:, :, 64:65], 1.0)
                    v_sb[b][s] = vt
                return step

            for s in range(8):
                steps.append(mk(s))
            return steps

        def attn_a(u, bsteps):
            b, j = divmod(u, 4)
            kt = qk[(b, "k", j)]
            qt = qk[(b, "q", j)]
            slot = 0
            nslots = 16
            for hh in range(2):
                mg = megap.tile([128, 8 * S], BF16, tag="mega", name=f"mega{u}_{hh}")
                mega[(u, hh)] = mg
                for sk in range(8):
                    ps = scp.tile([128, S], F32, tag="sc", name=f"sc{u}{hh}{sk}")
                    for sh in range(2):
                        nc.tensor.matmul(
                            ps[:, sh * 512:(sh + 1) * 512],
                            lhsT=kt[hh * 64: hh * 64 + 64, sk * 128:(sk + 1) * 128],
                            rhs=qt[hh * 64: hh * 64 + 64, sh * 512:(sh + 1) * 512],
                            start=True, stop=True,
                        )
                    if sk < DV:
                        nc.vector.scalar_tensor_tensor(
                            out=mg[:, sk * S:(sk + 1) * S],
                            in0=ps[:], scalar=SCH_A,
                            in1=masks[b][:, sk * S:(sk + 1) * S],
                            op0=mult, op1=mult,
                        )
                        if sk == DV - 1 or sk % 2 == 1:
                            lo = (sk // 2) * 2
                            hi = min(sk + 1, DV)
                            mgi = mg[:].bitcast(I16)
                            nc.vector.tensor_scalar(
                                mgi[:, lo * S:hi * S], mg[:, lo * S:hi * S],
                                1.0, SCH_B, mult, add)
                    else:
                        nc.scalar.activation(mg[:, sk * S:(sk + 1) * S], ps[:], EXP)
                        if sk == 7 or (sk - DV) % 2 == 1:
                            lo = max(DV, sk - ((sk - DV) % 2))
                            hi = sk + 1
                            nc.gpsimd.tensor_tensor(
                                out=mg[:, lo * S:hi * S],
                                in0=mg[:, lo * S:hi * S],
                                in1=masks[b][:, lo * S:hi * S],
                                op=powop,
                            )
                    slots_left = nslots - slot
                    n = (len(bsteps) + slots_left - 1) // slots_left if bsteps else 0
                    for _ in range(n):
                        if bsteps:
                            bsteps.pop(0)()
                    slot += 1
            for st in bsteps:
                st()

        def ctx_steps(u):
            b, j = divmod(u, 4)
            steps = []
            if b not in ctx_sb:
                ctx_sb[b] = ctxp.tile([128, 4096], BF16, tag="ctx", name=f"ctx{b}")

            pcs = {}

            def mk_chains(q4):
                def step():
                    pc = pcp.tile([128, 260], F32, tag="pc", name=f"pc{u}_{q4}")
                    pcs[q4] = pc
                    for i in range(2):
                        sq = q4 * 2 + i
                        for hh in range(2):
                            h = 2 * j + hh
                            mg = mega[(u, hh)]
                            for sk in range(8):
                                nc.tensor.matmul(
                                    pc[:, (i * 2 + hh) * 65:(i * 2 + hh) * 65 + 65],
                                    lhsT=mg[:, sk * S + sq * 128: sk * S + sq * 128 + 128],
                                    rhs=v_sb[b][sk][:, h * 65: h * 65 + 65],
                                    start=(sk == 0), stop=(sk == 7),
                                )
                return step

            def mk_norm(q4):
                def step():
                    pc = pcs[q4]
                    rcq = rcp.tile([128, 4], F32, tag="rc", name=f"rc{u}_{q4}")
                    pcv = pc[:].rearrange("p (g c) -> p g c", g=4)
                    nc.vector.reciprocal(rcq[:], pcv[:, :, 64])
                    base = rcq[0:128, 0:1]
                    in1 = bass.AP(tensor=base.tensor, offset=base.offset,
                                  ap=[base.ap[0], [2, 2], [1, 2], [0, 64]])
                    ov = ctx_sb[b][:].rearrange(
                        "p (sq h d) -> p sq h d", sq=8, h=8
                    )[:, q4 * 2:(q4 + 1) * 2, 2 * j:2 * j + 2, :]
                    nc.vector.scalar_tensor_tensor(
                        out=ov, in0=pcv[:, :, 0:64].rearrange("p (i hh) d -> p i hh d", i=2),
                        scalar=1.0, in1=in1, op0=mult, op1=mult)
                return step

            c = [mk_chains(q4) for q4 in range(4)]
            n = [mk_norm(q4) for q4 in range(4)]
            steps += [c[0], c[1], n[0], c[2], n[1], c[3], n[2], n[3]]
            return steps

        def transpose_steps(u):
            b, j = divmod(u, 4)
            steps = []
            if b not in ctxT:
                ctxT[b] = ctxTp.tile([128, 4096], BF16, tag="ctxT", name=f"ctxT{b}")

            def mk_tr(sq4):
                def step():
                    for sqi in range(4):
                        sq = sq4 * 4 + sqi
                        nc.sync.dma_start_transpose(
                            out=ctxT[b][:, j * S + sq * 128:(j * S + (sq + 1) * 128)],
                            in_=ctx_sb[b][:, sq * 512 + j * 128: sq * 512 + (j + 1) * 128],
                        )
                return step

            for sq4 in range(2):
                steps.append(mk_tr(sq4))
            return steps

        def outproj_steps(b, half):
            steps = []

            def mk(si):
                def step():
                    s = half * 4 + si
                    po = mmp.tile([128, 512], F32, tag="mm", name=f"po{b}_{s}")
                    for j4 in range(4):
                        nc.tensor.matmul(
                            po[:],
                            lhsT=ctxT[b][:, j4 * S + s * 128: j4 * S + (s + 1) * 128],
                            rhs=w_sb["o"][:, j4 * E:(j4 + 1) * E],
                            start=(j4 == 0), stop=(j4 == 3),
                        )
                    ou = outp.tile([128, 512], F32, tag="outs", name=f"ou{b}_{s}")
                    i = drain_ct["out"]
                    drain_ct["out"] += 1
                    if bo_nonzero:
                        nc.vector.scalar_tensor_tensor(
                            out=ou[:], in0=po[:], scalar=1.0, in1=bo_sb[:],
                            op0=mult, op1=add)
                    elif i % 16 < OUTDVE:
                        nc.vector.tensor_scalar(ou[:], po[:], 1.0, None, mult)
                    else:
                        nc.scalar.activation(ou[:], po[:], COPY)
                    nc.scalar.dma_start(
                        out=out_d[b, s * 128:(s + 1) * 128, :], in_=ou[:])
                return step

            for si in range(4):
                steps.append(mk(si))
            return steps

        # ---- emission ----
        load_w("q", wq_d, BF16)
        dma_in_x(0)
        load_w("k", wk_d, BF16)
        dma_in_mask(0, pieces=((0, 1), (1, 2), (2, 4), (4, 8)))
        load_w("v", wv_d, BF16)
        load_w("o", wo_d, BF16)

        for st in proj_qk_steps(0, 0):
            st()
        for st in proj_v_steps(0):
            st()

        for u in range(NU):
            b, j = divmod(u, 4)
            if u == 1 and BPC > 1:
                dma_in(1)
            bsteps = []
            if u + 1 < NU:
                nb, nj = divmod(u + 1, 4)
                bsteps += proj_qk_steps(nb, nj)
            if u == 3 and BPC > 1:
                bsteps += proj_v_steps(1)
            if u >= 1:
                bsteps += ctx_steps(u - 1)
            if u >= 2:
                bsteps += transpose_steps(u - 2)
            if u == 5:
                bsteps += outproj_steps(0, 0)
            if u == 6:
                bsteps += outproj_steps(0, 1)
            attn_a(u, bsteps)

        # tail: interleave ctx(7), transposes(6,7) and outproj(1) so no
        # engine sits behind a long serial chain.
        c7 = ctx_steps(NU - 1)  # [c0,c1,n0,c2,n1,c3,n2,n3]
        t6 = transpose_steps(NU - 2)
        t7 = transpose_steps(NU - 1)
        o0 = outproj_steps(BPC - 1, 0)
        o1 = outproj_steps(BPC - 1, 1)
        order = [c7[0], c7[1], t6[0], c7[2], c7[3], t6[1], c7[4], c7[5],
                 c7[6], c7[7], t7[0], o0[0], o0[1], t7[1], o0[2], o0[3]] + o1
        for st in order:
            st()

    nc.compile()
    return nc


def _prep(x, adj_matrix, bond_matrix, Wq, bq, Wk, bk, Wv, bv, Wo, bo):
    x = np.asarray(x, np.float32)
    mask = np.asarray(adj_matrix, np.float32) + np.asarray(bond_matrix, np.float32)
    xT = np.ascontiguousarray(x.transpose(0, 2, 1))
    xTb = xT.astype(NPBF16)
    maskT = np.ascontiguousarray(mask.transpose(0, 2, 1)).astype(NPBF16)
    wqT = np.ascontiguousarray(np.asarray(Wq, np.float32).T * SCALE).astype(NPBF16)
    wkT = np.ascontiguousarray(np.asarray(Wk, np.float32).T).astype(NPBF16)
    wvT = np.ascontiguousarray(np.asarray(Wv, np.float32).T).astype(NPBF16)
    woT = np.ascontiguousarray(np.asarray(Wo, np.float32).T).astype(NPBF16)
    bqs = np.asarray(bq, np.float32) * SCALE
    bkf = np.asarray(bk, np.float32)
    bqk = np.concatenate(
        [bqs.reshape(4, 128).T, bkf.reshape(4, 128).T], axis=1).astype(np.float32)
    bqk = np.ascontiguousarray(bqk)
    bvf = np.ascontiguousarray(np.asarray(bv, np.float32))
    bof = np.ascontiguousarray(np.asarray(bo, np.float32))

    in_maps = []
    for c in range(NCORES):
        sl = slice(c * BPC, (c + 1) * BPC)
        in_maps.append({
            "xT": np.ascontiguousarray(xTb[sl]),
            "maskT": np.ascontiguousarray(maskT[sl]),
            "wqT": wqT, "wkT": wkT, "wvT": wvT, "woT": woT,
            "bqk": bqk, "bv": bvf, "bo": bof,
        })
    return in_maps, bool(np.any(bof)), bool(np.any(bvf))


def kernel(x, adj_matrix, bond_matrix, Wq, bq, Wk, bk, Wv, bv, Wo, bo,
           seq_len, _trace=False, _knobs=None):
    in_maps, bo_nonzero, bv_nonzero = _prep(
        x, adj_matrix, bond_matrix, Wq, bq, Wk, bk, Wv, bv, Wo, bo)
    key = ("k", bo_nonzero, bv_nonzero, str(_knobs))
    if key not in _cache:
        _cache[key] = _build(bo_nonzero, bv_nonzero, _knobs)
    nc = _cache[key]
    res = run_bass_kernel_spmd(
        nc, in_maps, core_ids=list(range(NCORES)), trace=_trace)
    out = np.concatenate([r["out"] for r in res.results], axis=0).astype(np.float32)
    if _trace:
        kernel._last_exec_time_ns = res.exec_time_ns
        kernel._last_results = res
    return out
